# revision 17
# baseline (speedup 1.0000x reference)
"""Biased MHSA Trainium2 kernel (8-core SPMD), bf16 compute.

Sharding: core c -> (batch b = c//2, head-group g = c%2); each core computes
attention for 4 of the 8 heads of one batch and the partial output projection
for those heads. Host sums the two head-group partials per batch and adds
bo + bv @ wo.

Performance structure:
  - All matmuls in bf16 (PSUM accumulation stays fp32).
  - Bias handled as P = exp(S) * exp(bias): the host precomputes exp(bias)
    in bf16; the multiply runs on DVE in 2x bf16 mode from SBUF.
  - Score matmuls for a head pair use disjoint PE row-groups (contraction
    partitions 0-63 vs 64-127), so the hardware runs the pair concurrently
    (PE row tiling via auto tile_position); emitted sl-outer/i-inner so
    consecutive instructions pair up.
  - PV matmuls are software-pipelined one key-tile behind the scores so the
    in-order PE queue never waits on the exp->mult chain, which also keeps
    the PE dense enough that the HAM clock gate stays at 8/8 (2.4 GHz).
  - O-projection (head pairs stacked -> 128-wide contraction) for chunk q2-1
    runs in 8-tile blocks after each of chunk q2's head-pair passes.
  - Startup: 8 dummy matmuls pre-warm the HAM clock gate and a dummy exp
    preloads the ACT table while the split input DMAs land; exp(bias) tiles
    stream on the scalar/vector queues in parallel with the sync queue.
"""

import sys

if "/opt/trn_rl_repo" not in sys.path:
    sys.path.insert(0, "/opt/trn_rl_repo")

from contextlib import ExitStack

import numpy as np
import ml_dtypes

import concourse.bass as bass
from concourse import bacc
import concourse.tile as tile
from concourse import mybir
from concourse.bass_utils import run_bass_kernel_spmd

B, N, D = 4, 2048, 512
H, DH = 8, 64
HG = 4  # heads per core
GD = HG * DH  # 256 features per core
P = 128
QQ = 512  # matmul moving-dim slice
QW = 1024  # q processed in chunks of 1024
NQW = N // QW  # 2
NSL = QW // QQ  # 2
NKT = N // P  # 16 key tiles
KC = D // P  # 4 contraction chunks for projections
NN = N // QQ  # 4 token chunks for projections
F32 = mybir.dt.float32
F32R = mybir.dt.float32r
BF16 = mybir.dt.bfloat16
BF_NP = ml_dtypes.bfloat16


def build_program():
    nc = bacc.Bacc("TRN2", target_bir_lowering=False)
    xT = nc.dram_tensor("xT", [D, N], BF16, kind="ExternalInput")
    expbT = nc.dram_tensor("expbT", [N, N], BF16, kind="ExternalInput")
    wq = nc.dram_tensor("wq", [D, GD], BF16, kind="ExternalInput")
    wk = nc.dram_tensor("wk", [D, GD], BF16, kind="ExternalInput")
    wv = nc.dram_tensor("wv", [D, GD], BF16, kind="ExternalInput")
    wo = nc.dram_tensor("wo", [GD, D], BF16, kind="ExternalInput")
    bq = nc.dram_tensor("bq", [GD], F32, kind="ExternalInput")
    bk = nc.dram_tensor("bk", [GD], F32, kind="ExternalInput")
    out = nc.dram_tensor("out", [N, D], F32, kind="ExternalOutput")

    with tile.TileContext(nc) as tc, ExitStack() as ctx:
        const = ctx.enter_context(tc.tile_pool(name="const", bufs=1))
        big = ctx.enter_context(tc.tile_pool(name="big", bufs=1))
        p_pool = ctx.enter_context(tc.tile_pool(name="probs", bufs=6))
        small = ctx.enter_context(tc.tile_pool(name="small", bufs=2))
        o_pool = ctx.enter_context(tc.tile_pool(name="outp", bufs=6))
        psum_mm = ctx.enter_context(tc.tile_pool(name="psum_mm", bufs=2, space="PSUM"))
        psum_u = ctx.enter_context(tc.tile_pool(name="psum_u", bufs=1, space="PSUM"))

        # ---- HAM warm-up + ACT Exp-table preload (runs during input DMAs) --
        warm = const.tile([P, QQ], BF16)
        nc.vector.memset(warm, 0.0)
        expd = const.tile([P, 2], BF16)
        nc.scalar.activation(expd, warm[:, 0:2], mybir.ActivationFunctionType.Exp)
        wps = psum_mm.tile([P, QW], F32, tag="mm", name="warmps")
        for _ in range(8):
            nc.tensor.matmul(wps[:, 0:QQ], warm[:, 0:P], warm, start=True, stop=True)

        # ---- load inputs (split DMAs; sync queue feeds the projections,
        # scalar/vector queues stream the exp(bias) tiles in parallel) ----
        bq_s = const.tile([P, 2], F32)
        nc.sync.dma_start(out=bq_s, in_=bq.rearrange("(fc p) -> p fc", p=P))
        bk_s = const.tile([P, 2], F32)
        nc.sync.dma_start(out=bk_s, in_=bk.rearrange("(fc p) -> p fc", p=P))
        bqs = const.tile([P, 2], F32)  # bq * 0.125 (scale folded into Q)
        nc.vector.tensor_scalar_mul(bqs, bq_s, 0.125)
        wq_s = const.tile([P, KC, GD], BF16)
        wk_s = const.tile([P, KC, GD], BF16)
        wv_s = const.tile([P, KC, GD], BF16)
        xT_s = big.tile([P, KC, N], BF16)  # x^T as [128, kc, tok]
        for kc in range(KC):
            nc.sync.dma_start(out=wq_s[:, kc, :], in_=wq[kc * P : (kc + 1) * P, :])
        for nn in range(2):
            for kc in range(KC):
                nc.sync.dma_start(
                    out=xT_s[:, kc, nn * QQ : (nn + 1) * QQ],
                    in_=xT[kc * P : (kc + 1) * P, nn * QQ : (nn + 1) * QQ],
                )
        for kc in range(KC):
            nc.sync.dma_start(out=wk_s[:, kc, :], in_=wk[kc * P : (kc + 1) * P, :])
        for nn in range(2, NN):
            for kc in range(KC):
                nc.sync.dma_start(
                    out=xT_s[:, kc, nn * QQ : (nn + 1) * QQ],
                    in_=xT[kc * P : (kc + 1) * P, nn * QQ : (nn + 1) * QQ],
                )
        for kc in range(KC):
            nc.sync.dma_start(out=wv_s[:, kc, :], in_=wv[kc * P : (kc + 1) * P, :])
        wo_s = const.tile([P, 2, D], BF16)  # head-pair wo rows: [128, hp, 512]
        nc.sync.dma_start(out=wo_s, in_=wo.rearrange("(g p) d -> p g d", p=P))
        # exp(bias)^T tiles, bf16, loaded once upfront on the side queues
        ebt = big.tile([P, NQW, NKT, QW], BF16)
        for kt in range(NKT):
            nc.scalar.dma_start(
                out=ebt[:, 0, kt, :],
                in_=expbT[kt * P : (kt + 1) * P, 0:QW],
            )
        for kt in range(NKT):
            nc.scalar.dma_start(
                out=ebt[:, 1, kt, :],
                in_=expbT[kt * P : (kt + 1) * P, QW : 2 * QW],
            )

        # ---- projections ----
        # Q^T, K^T: [128, fc, tok] (feature on partitions; head-pair fc has
        # head 2fc on partitions 0..63 and head 2fc+1 on 64..127)
        qT = big.tile([P, 2, N], BF16)
        kT = big.tile([P, 2, N], BF16)
        vaug = big.tile([P, HG, NKT, DH + 1], BF16)  # [tok, h, kt, 64 V | 1]
        nc.vector.memset(vaug[:, :, :, DH : DH + 1], 1.0)
        ones64 = const.tile([1, DH], BF16)  # for the 1/r PE broadcast
        nc.vector.memset(ones64, 1.0)
        for w_s, dst, b_ap, scale in (
            (wq_s, qT, bqs, 0.125),
            (wk_s, kT, bk_s, 1.0),
        ):
            for nn in range(NN):
                for fc in range(2):
                    ps = psum_mm.tile([P, QW], F32, tag="mm", name="ps_p")
                    for kc in range(KC):
                        nc.tensor.matmul(
                            ps[:, 0:QQ],
                            (w_s[:, kc, fc * P : (fc + 1) * P]),
                            (xT_s[:, kc, nn * QQ : (nn + 1) * QQ]),
                            start=(kc == 0),
                            stop=(kc == KC - 1),
                        )
                    # (x@w + b) * scale  ==  psum*scale + b*scale
                    nc.vector.tensor_scalar(
                        dst[:, fc, nn * QQ : (nn + 1) * QQ],
                        ps[:, 0:QQ],
                        scale,
                        b_ap[:, fc : fc + 1],
                        op0=mybir.AluOpType.mult,
                        op1=mybir.AluOpType.add,
                    )
        for kt in range(NKT):  # V natural layout (bv folded on host)
            ps = psum_mm.tile([P, QW], F32, tag="mm", name="ps_v")
            for kc in range(KC):
                nc.tensor.matmul(
                    ps[:, 0:GD],
                    (xT_s[:, kc, kt * P : (kt + 1) * P]),
                    (wv_s[:, kc, :]),
                    start=(kc == 0),
                    stop=(kc == KC - 1),
                )
            nc.vector.tensor_copy(
                vaug[:, :, kt, 0:DH],
                ps[:, 0:GD].rearrange("p (h d) -> p h d", h=HG),
            )

        # ---- attention ----
        # A^T (normalized attn out), head pair stacked on 128 partitions:
        # head 2hp+i at partitions i*64..(i+1)*64 of chunk hp.
        aT = big.tile([P, 2, N], BF16)

        def normalize_head(hp, i, pu_h, qsl):
            # Evict U_aug^T fast (releases the PSUM banks for the next pass).
            # 1/r is broadcast across the 64 feature partitions by a tiny PE
            # matmul (ones64^T @ r_inv) into the just-freed pu slot -- no DRAM
            # round trip, so the chain can't be starved by output-DMA bursts.
            uT = small.tile([DH + 1, QW], F32, tag="uT", name="uT")
            nc.vector.tensor_copy(uT, pu_h)
            r_inv = small.tile([1, QW], BF16, tag="rinv", name="r_inv")
            with nc.allow_low_precision(reason="bf16 1/r: tolerance is 2e-2"):
                nc.vector.reciprocal(r_inv, uT[DH : DH + 1, :])
            bcp = psum_u.tile([DH, QW], F32, tag=f"u{i}", name="bcp")
            for sl in range(NSL):
                ssl = slice(sl * QQ, (sl + 1) * QQ)
                nc.tensor.matmul(
                    bcp[:, ssl], ones64, r_inv[:, ssl], start=True, stop=True
                )
            nc.vector.tensor_tensor(
                aT[i * DH : (i + 1) * DH, hp, qsl],
                uT[0:DH, :],
                bcp,
                op=mybir.AluOpType.mult,
            )

        def oproj_tile(t):
            # O[tok, 512] = sum_hp A_pair^T.T @ wo_pair for one token tile
            ps = psum_mm.tile([P, QW], F32, tag="mm", name="ps_o")
            for hp in range(2):
                nc.tensor.matmul(
                    ps[:, 0:D],
                    aT[:, hp, t * P : (t + 1) * P],
                    wo_s[:, hp, :],
                    start=(hp == 0),
                    stop=(hp == 1),
                )
            ob = o_pool.tile([P, D], F32, name="ob")
            nc.vector.tensor_copy(ob, ps[:, 0:D])
            nc.sync.dma_start(out=out[t * P : (t + 1) * P, :], in_=ob)

        for q2 in range(NQW):  # q2 outer; O-proj of q2-1 interleaves into q2
            qsl = slice(q2 * QW, (q2 + 1) * QW)
            for hp in range(2):  # head pair: heads (2hp, 2hp+1) live in fc=hp
                pu = [
                    psum_u.tile([DH + 1, QW], F32, tag=f"u{i}", name=f"pu{i}")
                    for i in range(2)
                ]

                def emit_pv(j, sps):
                    for i in range(2):
                        for sl in range(NSL):
                            ssl = slice(sl * QQ, (sl + 1) * QQ)
                            nc.tensor.matmul(
                                pu[i][:, ssl],
                                vaug[:, 2 * hp + i, j, :],
                                sps[i][:, ssl],
                                start=(j == 0),
                                stop=(j == NKT - 1),
                            )

                sp_prev = None
                for kt in range(NKT):
                    # scores for both heads; sl-outer/i-inner so the i=0/i=1
                    # matmuls (disjoint PE row groups) run concurrently
                    ps = [
                        psum_mm.tile([P, QW], F32, tag="mm", name=f"ps{i}")
                        for i in range(2)
                    ]
                    for sl in range(NSL):
                        ssl = slice(sl * QQ, (sl + 1) * QQ)
                        for i in range(2):
                            ho = i * DH
                            nc.tensor.matmul(
                                ps[i][:, ssl],
                                kT[ho : ho + DH, hp, kt * P : (kt + 1) * P],
                                qT[ho : ho + DH, hp, q2 * QW + sl * QQ :
                                   q2 * QW + (sl + 1) * QQ],
                                start=True,
                                stop=True,
                            )
                    sp_cur = []
                    for i in range(2):
                        sp = p_pool.tile([P, QW], BF16, tag="sp", name="sp")
                        nc.scalar.activation(
                            sp, ps[i], mybir.ActivationFunctionType.Exp
                        )
                        # P = exp(S) * exp(bias): 2x bf16 DVE mode, in place
                        nc.vector.tensor_tensor(
                            sp, sp, ebt[:, q2, kt, :], op=mybir.AluOpType.mult
                        )
                        sp_cur.append(sp)
                    if kt > 0:
                        emit_pv(kt - 1, sp_prev)
                    sp_prev = sp_cur
                emit_pv(NKT - 1, sp_prev)
                for i in range(2):
                    normalize_head(hp, i, pu[i], qsl)
                if q2 > 0:
                    # O-proj of the previous q chunk (8 tiles, aT deps long
                    # resolved), one 4-tile block per hp pass
                    for t in range(4):
                        oproj_tile((q2 - 1) * (QW // P) + hp * 4 + t)
        for t in range((NQW - 1) * QW // P, NQW * QW // P):
            oproj_tile(t)

    nc.compile()
    return nc


_NC = None


def _get_nc():
    global _NC
    if _NC is None:
        _NC = build_program()
    return _NC


def make_in_maps(x, attn_bias, wq, bq, wk, bk, wv, bv, wo, bo):
    x = np.asarray(x, np.float32)
    attn_bias = np.asarray(attn_bias, np.float32)
    expbT_b = [
        np.ascontiguousarray(np.exp(attn_bias[b, 0]).T.astype(BF_NP))
        for b in range(B)
    ]
    xT_b = [np.ascontiguousarray(x[b].T.astype(BF_NP)) for b in range(B)]
    wq = np.asarray(wq, np.float32)
    wk = np.asarray(wk, np.float32)
    wv = np.asarray(wv, np.float32)
    wo = np.asarray(wo, np.float32)
    in_maps = []
    for c in range(8):
        b, g = c // 2, c % 2
        sl = slice(g * GD, (g + 1) * GD)
        in_maps.append(
            {
                "xT": xT_b[b],
                "expbT": expbT_b[b],
                "wq": np.ascontiguousarray(wq[:, sl].astype(BF_NP)),
                "wk": np.ascontiguousarray(wk[:, sl].astype(BF_NP)),
                "wv": np.ascontiguousarray(wv[:, sl].astype(BF_NP)),
                "wo": np.ascontiguousarray(wo[sl, :].astype(BF_NP)),
                "bq": np.ascontiguousarray(np.asarray(bq, np.float32)[sl]),
                "bk": np.ascontiguousarray(np.asarray(bk, np.float32)[sl]),
            }
        )
    return in_maps


def gather_output(results, bo, bv, wo):
    bo = np.asarray(bo, np.float32)
    row = bo + np.asarray(bv, np.float32) @ np.asarray(wo, np.float32)
    out = np.empty((B, N, D), np.float32)
    for b in range(B):
        out[b] = results[2 * b]["out"] + results[2 * b + 1]["out"] + row[None, :]
    return out


def kernel(x, attn_bias, wq, bq, wk, bk, wv, bv, wo, bo, _trace=False):
    nc = _get_nc()
    in_maps = make_in_maps(x, attn_bias, wq, bq, wk, bk, wv, bv, wo, bo)
    res = run_bass_kernel_spmd(nc, in_maps, core_ids=list(range(8)), trace=_trace)
    out = gather_output(res.results, bo, bv, wo)
    if _trace:
        kernel.last_results = res
    return out


# revision 21
# speedup vs baseline: 1.3039x; 1.3039x over previous
"""Biased MHSA Trainium2 kernel (8-core SPMD), bf16 compute.

Sharding: core c -> (batch b = c//2, head-group g = c%2); each core computes
attention for 4 of the 8 heads of one batch and the partial output projection
for those heads. Host sums the two head-group partials per batch and adds
bo + bv @ wo.

Performance structure:
  - All matmuls in bf16 (PSUM accumulation stays fp32).
  - Bias handled as P = exp(S) * exp(bias): the host precomputes exp(bias)
    in bf16; the multiply runs on DVE in 2x bf16 mode from SBUF.
  - Score matmuls for a head pair use disjoint PE row-groups (contraction
    partitions 0-63 vs 64-127), so the hardware runs the pair concurrently
    (PE row tiling via auto tile_position); emitted sl-outer/i-inner so
    consecutive instructions pair up.
  - PV matmuls are software-pipelined one key-tile behind the scores so the
    in-order PE queue never waits on the exp->mult chain, which also keeps
    the PE dense enough that the HAM clock gate stays at 8/8 (2.4 GHz).
  - O-projection (head pairs stacked -> 128-wide contraction) for chunk q2-1
    runs in 8-tile blocks after each of chunk q2's head-pair passes.
  - Startup: 8 dummy matmuls pre-warm the HAM clock gate and a dummy exp
    preloads the ACT table while the split input DMAs land; exp(bias) tiles
    stream on the scalar/vector queues in parallel with the sync queue.
"""

import sys

if "/opt/trn_rl_repo" not in sys.path:
    sys.path.insert(0, "/opt/trn_rl_repo")

from contextlib import ExitStack

import numpy as np
import ml_dtypes

import concourse.bass as bass
from concourse import bacc
import concourse.tile as tile
from concourse import mybir
from concourse.bass_utils import run_bass_kernel_spmd

B, N, D = 4, 2048, 512
H, DH = 8, 64
HG = 4  # heads per core
GD = HG * DH  # 256 features per core
P = 128
QQ = 512  # matmul moving-dim slice
QW = 1024  # q processed in chunks of 1024
NQW = N // QW  # 2
NSL = QW // QQ  # 2
NKT = N // P  # 16 key tiles
KC = D // P  # 4 contraction chunks for projections
NN = N // QQ  # 4 token chunks for projections
F32 = mybir.dt.float32
F32R = mybir.dt.float32r
BF16 = mybir.dt.bfloat16
BF_NP = ml_dtypes.bfloat16


def build_program():
    nc = bacc.Bacc("TRN2", target_bir_lowering=False)
    xT = nc.dram_tensor("xT", [D, N], BF16, kind="ExternalInput")
    expbT = nc.dram_tensor("expbT", [N, N], BF16, kind="ExternalInput")
    wq = nc.dram_tensor("wq", [D, GD], BF16, kind="ExternalInput")
    wk = nc.dram_tensor("wk", [D, GD], BF16, kind="ExternalInput")
    wv = nc.dram_tensor("wv", [D, GD], BF16, kind="ExternalInput")
    wo = nc.dram_tensor("wo", [GD, D], BF16, kind="ExternalInput")
    bq = nc.dram_tensor("bq", [GD], F32, kind="ExternalInput")
    bk = nc.dram_tensor("bk", [GD], F32, kind="ExternalInput")
    out = nc.dram_tensor("out", [N, D], F32, kind="ExternalOutput")

    with tile.TileContext(nc) as tc, ExitStack() as ctx:
        const = ctx.enter_context(tc.tile_pool(name="const", bufs=1))
        big = ctx.enter_context(tc.tile_pool(name="big", bufs=1))
        p_pool = ctx.enter_context(tc.tile_pool(name="probs", bufs=6))
        small = ctx.enter_context(tc.tile_pool(name="small", bufs=2))
        o_pool = ctx.enter_context(tc.tile_pool(name="outp", bufs=6))
        psum_mm = ctx.enter_context(tc.tile_pool(name="psum_mm", bufs=2, space="PSUM"))
        psum_u = ctx.enter_context(tc.tile_pool(name="psum_u", bufs=1, space="PSUM"))
        dram_p = ctx.enter_context(tc.tile_pool(name="dram_p", bufs=2, space="DRAM"))

        # ---- HAM warm-up + ACT Exp-table preload (runs during input DMAs) --
        warm = const.tile([P, QQ], BF16)
        nc.vector.memset(warm, 0.0)
        expd = const.tile([P, 2], BF16)
        nc.scalar.activation(expd, warm[:, 0:2], mybir.ActivationFunctionType.Exp)
        wps = psum_mm.tile([P, QW], F32, tag="mm", name="warmps")
        for _ in range(8):
            nc.tensor.matmul(wps[:, 0:QQ], warm[:, 0:P], warm, start=True, stop=True)

        # ---- load inputs (split DMAs; sync queue feeds the projections,
        # scalar/vector queues stream the exp(bias) tiles in parallel) ----
        bq_s = const.tile([P, 2], F32)
        nc.sync.dma_start(out=bq_s, in_=bq.rearrange("(fc p) -> p fc", p=P))
        bk_s = const.tile([P, 2], F32)
        nc.sync.dma_start(out=bk_s, in_=bk.rearrange("(fc p) -> p fc", p=P))
        bqs = const.tile([P, 2], F32)  # bq * 0.125 (scale folded into Q)
        nc.vector.tensor_scalar_mul(bqs, bq_s, 0.125)
        wq_s = const.tile([P, KC, GD], BF16)
        wk_s = const.tile([P, KC, GD], BF16)
        wv_s = const.tile([P, KC, GD], BF16)
        xT_s = big.tile([P, KC, N], BF16)  # x^T as [128, kc, tok]
        for kc in range(KC):
            nc.sync.dma_start(out=wq_s[:, kc, :], in_=wq[kc * P : (kc + 1) * P, :])
        for nn in range(2):
            for kc in range(KC):
                nc.sync.dma_start(
                    out=xT_s[:, kc, nn * QQ : (nn + 1) * QQ],
                    in_=xT[kc * P : (kc + 1) * P, nn * QQ : (nn + 1) * QQ],
                )
        for kc in range(KC):
            nc.sync.dma_start(out=wk_s[:, kc, :], in_=wk[kc * P : (kc + 1) * P, :])
        for nn in range(2, NN):
            for kc in range(KC):
                nc.sync.dma_start(
                    out=xT_s[:, kc, nn * QQ : (nn + 1) * QQ],
                    in_=xT[kc * P : (kc + 1) * P, nn * QQ : (nn + 1) * QQ],
                )
        for kc in range(KC):
            nc.sync.dma_start(out=wv_s[:, kc, :], in_=wv[kc * P : (kc + 1) * P, :])
        wo_s = const.tile([P, 2, D], BF16)  # head-pair wo rows: [128, hp, 512]
        nc.sync.dma_start(out=wo_s, in_=wo.rearrange("(g p) d -> p g d", p=P))
        # exp(bias)^T tiles, bf16, loaded once upfront on the side queues
        ebt = big.tile([P, NQW, NKT, QW], BF16)
        for kt in range(NKT):
            nc.scalar.dma_start(
                out=ebt[:, 0, kt, :],
                in_=expbT[kt * P : (kt + 1) * P, 0:QW],
            )
        for kt in range(NKT):
            nc.scalar.dma_start(
                out=ebt[:, 1, kt, :],
                in_=expbT[kt * P : (kt + 1) * P, QW : 2 * QW],
            )

        # ---- projections ----
        # Q^T, K^T: [128, fc, tok] (feature on partitions; head-pair fc has
        # head 2fc on partitions 0..63 and head 2fc+1 on 64..127)
        qT = big.tile([P, 2, N], BF16)
        kT = big.tile([P, 2, N], BF16)
        vaug = big.tile([P, HG, NKT, DH + 1], BF16)  # [tok, h, kt, 64 V | 1]
        nc.vector.memset(vaug[:, :, :, DH : DH + 1], 1.0)

        for w_s, dst, b_ap, scale in (
            (wq_s, qT, bqs, 0.125),
            (wk_s, kT, bk_s, 1.0),
        ):
            for nn in range(NN):
                for fc in range(2):
                    ps = psum_mm.tile([P, QW], F32, tag="mm", name="ps_p")
                    for kc in range(KC):
                        nc.tensor.matmul(
                            ps[:, 0:QQ],
                            (w_s[:, kc, fc * P : (fc + 1) * P]),
                            (xT_s[:, kc, nn * QQ : (nn + 1) * QQ]),
                            start=(kc == 0),
                            stop=(kc == KC - 1),
                        )
                    # (x@w + b) * scale  ==  psum*scale + b*scale
                    nc.vector.tensor_scalar(
                        dst[:, fc, nn * QQ : (nn + 1) * QQ],
                        ps[:, 0:QQ],
                        scale,
                        b_ap[:, fc : fc + 1],
                        op0=mybir.AluOpType.mult,
                        op1=mybir.AluOpType.add,
                    )
        for kt in range(NKT):  # V natural layout (bv folded on host)
            ps = psum_mm.tile([P, QW], F32, tag="mm", name="ps_v")
            for kc in range(KC):
                nc.tensor.matmul(
                    ps[:, 0:GD],
                    (xT_s[:, kc, kt * P : (kt + 1) * P]),
                    (wv_s[:, kc, :]),
                    start=(kc == 0),
                    stop=(kc == KC - 1),
                )
            nc.vector.tensor_copy(
                vaug[:, :, kt, 0:DH],
                ps[:, 0:GD].rearrange("p (h d) -> p h d", h=HG),
            )

        # ---- attention ----
        # A^T (normalized attn out), head pair stacked on 128 partitions:
        # head 2hp+i at partitions i*64..(i+1)*64 of chunk hp.
        aT = big.tile([P, 2, N], BF16)

        def normalize_head(hp, i, pu_h, qsl):
            # Evict U_aug^T fast (releases the PSUM banks for the next pass);
            # normalization runs off the critical path from SBUF. The r row is
            # respread to [128, 8] via a DRAM bounce so reciprocal is cheap,
            # then broadcast across 64 partitions with a 0-stride DMA.
            uT = small.tile([DH + 1, QW], F32, tag="uT", name="uT")
            nc.vector.tensor_copy(uT, pu_h)
            r_d = dram_p.tile([QW], F32, tag="rd", name="r_d")
            nc.gpsimd.dma_start(out=r_d[:], in_=uT[DH : DH + 1, :])
            r128 = small.tile([P, QW // P], F32, tag="r128", name="r128")
            nc.gpsimd.dma_start(out=r128, in_=r_d[:].rearrange("(f p) -> p f", p=P))
            nc.vector.reciprocal(r128, r128)
            rd2 = dram_p.tile([QW], F32, tag="rd2", name="rd2")
            nc.gpsimd.dma_start(out=rd2[:].rearrange("(f p) -> p f", p=P), in_=r128)
            bc = small.tile([DH, QW], F32, tag="bc", name="bc")
            rap = rd2[:]
            nc.gpsimd.dma_start(
                out=bc,
                in_=bass.AP(
                    tensor=rap.tensor, offset=rap.offset,
                    ap=[[0, DH]] + list(rap.ap),
                ),
            )
            nc.vector.tensor_tensor(
                aT[i * DH : (i + 1) * DH, hp, qsl],
                uT[0:DH, :],
                bc,
                op=mybir.AluOpType.mult,
            )

        def oproj_tile(t):
            # O[tok, 512] = sum_hp A_pair^T.T @ wo_pair for one token tile
            ps = psum_mm.tile([P, QW], F32, tag="mm", name="ps_o")
            for hp in range(2):
                nc.tensor.matmul(
                    ps[:, 0:D],
                    aT[:, hp, t * P : (t + 1) * P],
                    wo_s[:, hp, :],
                    start=(hp == 0),
                    stop=(hp == 1),
                )
            ob = o_pool.tile([P, D], F32, name="ob")
            nc.vector.tensor_copy(ob, ps[:, 0:D])
            nc.sync.dma_start(out=out[t * P : (t + 1) * P, :], in_=ob)

        for q2 in range(NQW):  # q2 outer; O-proj of q2-1 interleaves into q2
            qsl = slice(q2 * QW, (q2 + 1) * QW)
            for hp in range(2):  # head pair: heads (2hp, 2hp+1) live in fc=hp
                pu = [
                    psum_u.tile([DH + 1, QW], F32, tag=f"u{i}", name=f"pu{i}")
                    for i in range(2)
                ]

                def emit_pv(j, sps):
                    for i in range(2):
                        for sl in range(NSL):
                            ssl = slice(sl * QQ, (sl + 1) * QQ)
                            nc.tensor.matmul(
                                pu[i][:, ssl],
                                vaug[:, 2 * hp + i, j, :],
                                sps[i][:, ssl],
                                start=(j == 0),
                                stop=(j == NKT - 1),
                            )

                sp_prev = None
                for kt in range(NKT):
                    # scores for both heads; sl-outer/i-inner so the i=0/i=1
                    # matmuls (disjoint PE row groups) run concurrently
                    ps = [
                        psum_mm.tile([P, QW], F32, tag="mm", name=f"ps{i}")
                        for i in range(2)
                    ]
                    for sl in range(NSL):
                        ssl = slice(sl * QQ, (sl + 1) * QQ)
                        for i in range(2):
                            ho = i * DH
                            nc.tensor.matmul(
                                ps[i][:, ssl],
                                kT[ho : ho + DH, hp, kt * P : (kt + 1) * P],
                                qT[ho : ho + DH, hp, q2 * QW + sl * QQ :
                                   q2 * QW + (sl + 1) * QQ],
                                start=True,
                                stop=True,
                            )
                    sp_cur = []
                    for i in range(2):
                        sp = p_pool.tile([P, QW], BF16, tag="sp", name="sp")
                        nc.scalar.activation(
                            sp, ps[i], mybir.ActivationFunctionType.Exp
                        )
                        # P = exp(S) * exp(bias): 2x bf16 DVE mode, in place
                        nc.vector.tensor_tensor(
                            sp, sp, ebt[:, q2, kt, :], op=mybir.AluOpType.mult
                        )
                        sp_cur.append(sp)
                    if kt > 0:
                        emit_pv(kt - 1, sp_prev)
                    sp_prev = sp_cur
                emit_pv(NKT - 1, sp_prev)
                for i in range(2):
                    normalize_head(hp, i, pu[i], qsl)
                if q2 > 0:
                    # O-proj of the previous q chunk (8 tiles, aT deps long
                    # resolved), one 4-tile block per hp pass
                    for t in range(4):
                        oproj_tile((q2 - 1) * (QW // P) + hp * 4 + t)
        for t in range((NQW - 1) * QW // P, NQW * QW // P):
            oproj_tile(t)

    nc.compile()
    return nc


_NC = None


def _get_nc():
    global _NC
    if _NC is None:
        _NC = build_program()
    return _NC


def make_in_maps(x, attn_bias, wq, bq, wk, bk, wv, bv, wo, bo):
    x = np.asarray(x, np.float32)
    attn_bias = np.asarray(attn_bias, np.float32)
    expbT_b = [
        np.ascontiguousarray(np.exp(attn_bias[b, 0]).T.astype(BF_NP))
        for b in range(B)
    ]
    xT_b = [np.ascontiguousarray(x[b].T.astype(BF_NP)) for b in range(B)]
    wq = np.asarray(wq, np.float32)
    wk = np.asarray(wk, np.float32)
    wv = np.asarray(wv, np.float32)
    wo = np.asarray(wo, np.float32)
    in_maps = []
    for c in range(8):
        b, g = c // 2, c % 2
        sl = slice(g * GD, (g + 1) * GD)
        in_maps.append(
            {
                "xT": xT_b[b],
                "expbT": expbT_b[b],
                "wq": np.ascontiguousarray(wq[:, sl].astype(BF_NP)),
                "wk": np.ascontiguousarray(wk[:, sl].astype(BF_NP)),
                "wv": np.ascontiguousarray(wv[:, sl].astype(BF_NP)),
                "wo": np.ascontiguousarray(wo[sl, :].astype(BF_NP)),
                "bq": np.ascontiguousarray(np.asarray(bq, np.float32)[sl]),
                "bk": np.ascontiguousarray(np.asarray(bk, np.float32)[sl]),
            }
        )
    return in_maps


def gather_output(results, bo, bv, wo):
    bo = np.asarray(bo, np.float32)
    row = bo + np.asarray(bv, np.float32) @ np.asarray(wo, np.float32)
    out = np.empty((B, N, D), np.float32)
    for b in range(B):
        out[b] = results[2 * b]["out"] + results[2 * b + 1]["out"] + row[None, :]
    return out


def kernel(x, attn_bias, wq, bq, wk, bk, wv, bv, wo, bo, _trace=False):
    nc = _get_nc()
    in_maps = make_in_maps(x, attn_bias, wq, bq, wk, bk, wv, bv, wo, bo)
    res = run_bass_kernel_spmd(nc, in_maps, core_ids=list(range(8)), trace=_trace)
    out = gather_output(res.results, bo, bv, wo)
    if _trace:
        kernel.last_results = res
    return out
